# revision 2
# baseline (speedup 1.0000x reference)
"""Trainium2 Bass kernel for nn_CombinedRotaryEmbedding.

Math
----
reference(x, ...) does, per (batch, seq, head) row r of length 64:
  1. 32 sequential Givens plane rotations -> r @ M_0 @ ... @ M_31
  2. r @ r_matrix
  3. RoPE mix with per-position sin/cos over even/odd channel pairs.

Steps 1-2 fold into ONE 64x64 matrix  Gtot = M_0 @ ... @ M_31 @ r_matrix
(computed on host in float64 from the tiny params).  We further permute
Gtot's columns (evens first, odds second) so that after  y1 = x @ Gp  the
head layout is [u | v] with u = x1 (even channels), v = x2 (odd channels)
— which is exactly the reference's output channel layout:
  out[..., 0:32]  = u*cos - v*sin
  out[..., 32:64] = u*sin + v*cos
The second RoPE operand  y2 = [-v | u]  is a sign-flipped half-swap of
y1, so no second matmul is needed: the elementwise stage reads y1 twice,
once with a half-swapped access pattern and a sign-folded sin table.

Device pipeline (per 128-token tile, fully unrolled, 32 tiles/core):
  ACT-DGE: DMA in x[128,1024]  ->  PE: 8x transpose(128x128) -> PSUM
  ACT: copy PSUM->SBUF  ->  PE: 8x matmul(lhsT=xT_blk, rhs=blockdiag(Gp,Gp))
  DVE: t1 = y1 * cos_bcast ; t2 = y1 * sin_signed_bcast   (PSUM src)
  POOL: out = t1 + half_swap(t2)   ->  SP-DGE: DMA out.
The full [4096, 128] cos|sin table is DMA'd to SBUF once at start.

Sharding: data-parallel, batch b -> core b (8 batches, 8 cores); all
params tiny and replicated.  Positions per core are s = 0..4095, so one
trig table is shared by all cores.
"""

import numpy as np

import concourse.bass as bass
import concourse.tile as tile
from concourse import bacc, mybir
from concourse.bass_utils import run_bass_kernel_spmd

# Problem constants (hardcoded per the task contract).
B, S, N_STATE, N_HEAD = 8, 4096, 1024, 16
H_DIM = N_STATE // N_HEAD        # 64
HALF = H_DIM // 2                # 32
N_CORES = 8
P = 128                          # partitions / tokens per tile
TOKENS_PER_CORE = S              # 4096
N_TILES = TOKENS_PER_CORE // P   # 32
N_BLK = N_STATE // P             # 8 channel blocks of 128 (2 heads each)

_BUILD_CACHE = {}


def _fold_g(angles, r_pairs, r_matrix):
    """Fold the Givens scan + r_matrix into one 64x64 (float64)."""
    g = np.eye(H_DIM, dtype=np.float64)
    eye = np.eye(H_DIM, dtype=np.float64)
    for k in range(angles.shape[0]):
        i, j = int(r_pairs[k, 0]), int(r_pairs[k, 1])
        c, sn = np.cos(angles[k]), np.sin(angles[k])
        m = eye.copy()
        # column i then column j, from the ORIGINAL basis columns —
        # replicates the reference's read-before-write .at[].set order
        # (also correct if i == j: the j write overwrites the i write).
        m[:, i] = c * eye[:, i] + sn * eye[:, j]
        m[:, j] = -sn * eye[:, i] + c * eye[:, j]
        g = g @ m
    g = g @ np.asarray(r_matrix, np.float64)
    return g


def _build_constants(thetas, theta_scale, r_matrix, inv_freq, r_pairs):
    """Host-side constant folding.

    Matches the reference's fp32 quantization points: the angle products
    (thetas * theta_scale, pos * inv_freq) are rounded to fp32 before the
    trig, as the fp32 reference does.
    """
    thetas = np.asarray(thetas, np.float32)
    theta_scale = np.asarray(theta_scale, np.float32)
    r_matrix = np.asarray(r_matrix, np.float32)
    inv_freq = np.asarray(inv_freq, np.float32)

    angles = (thetas * theta_scale[0]).astype(np.float32).astype(np.float64)
    gtot = _fold_g(angles, np.asarray(r_pairs), r_matrix)

    # Column permutation: evens first then odds -> y1 = [u | v] per head.
    perm = np.concatenate([np.arange(0, H_DIM, 2), np.arange(1, H_DIM, 2)])
    gp = gtot[:, perm].astype(np.float32)
    gp2 = np.zeros((P, P), np.float32)
    gp2[:H_DIM, :H_DIM] = gp
    gp2[H_DIM:, H_DIM:] = gp

    ident = np.eye(P, dtype=np.float32)

    pos = np.arange(S, dtype=np.float32)
    sinu32 = (pos[:, None] * inv_freq[None, :]).astype(np.float32)
    s64 = sinu32.astype(np.float64)
    cos_t = np.cos(s64).astype(np.float32)  # [S, 32]
    sin_t = np.sin(s64).astype(np.float32)
    # trig row layout per position: [cos|cos | +sin|-sin]  (128 wide).
    # cosd = [cos|cos]; sinds = [+sin|-sin]: t2 is computed in y1's layout
    # and later read half-swapped, so the coefficient that must land on
    # output half 0 (-sin) is stored in half 1 and vice versa.
    trig = np.concatenate([cos_t, cos_t, sin_t, -sin_t], axis=1)  # [S, 128]
    return gp2, ident, trig


def _build_program():
    """Build + compile the per-core Bass program (same NEFF on all cores)."""
    nc = bacc.Bacc("TRN2", target_bir_lowering=False, debug=False,
                   num_devices=N_CORES)
    dt = mybir.dt.float32

    x = nc.dram_tensor("x", [TOKENS_PER_CORE, N_STATE], dt,
                       kind="ExternalInput").ap()
    gp2 = nc.dram_tensor("gp2", [P, P], dt, kind="ExternalInput").ap()
    ident = nc.dram_tensor("ident", [P, P], dt, kind="ExternalInput").ap()
    trig = nc.dram_tensor("trig", [S, P], dt, kind="ExternalInput").ap()
    out = nc.dram_tensor("out", [TOKENS_PER_CORE, N_STATE], dt,
                         kind="ExternalOutput").ap()

    with tile.TileContext(nc) as tc:
        with (
            tc.tile_pool(name="const", bufs=1) as cpool,
            tc.tile_pool(name="xin", bufs=4) as xpool,
            tc.tile_pool(name="xt", bufs=3) as xtpool,
            tc.tile_pool(name="mix", bufs=3) as mixpool,
            tc.tile_pool(name="outp", bufs=4) as opool,
            tc.tile_pool(name="ps_xt", bufs=2, space="PSUM") as ps_xt,
            tc.tile_pool(name="ps_y", bufs=2, space="PSUM") as ps_y,
        ):
            gp2_sb = cpool.tile([P, P], dt, tag="gp2")
            id_sb = cpool.tile([P, P], dt, tag="ident")
            nc.sync.dma_start(gp2_sb[:], gp2)
            nc.sync.dma_start(id_sb[:], ident)

            # Whole trig table, one DMA: tile t lives at columns 128t..128t+127.
            trig_sb = cpool.tile([P, N_TILES * P], dt, tag="trig")
            trig_dst = trig_sb[:].rearrange("p (t w) -> p t w", w=P)
            trig_src = trig.rearrange("(t p) w -> p t w", p=P)
            nc.sync.dma_start(trig_dst, trig_src)

            for t in range(N_TILES):
                rows = slice(t * P, (t + 1) * P)

                x_sb = xpool.tile([P, N_STATE], dt, tag="x")
                nc.scalar.dma_start(x_sb[:], x[rows, :])

                # Transpose 8 channel blocks: xt[c, tok] for c in block b.
                xt_p = ps_xt.tile([P, N_STATE], dt, tag="xt_ps")
                for b in range(N_BLK):
                    cols = slice(b * P, (b + 1) * P)
                    nc.tensor.transpose(xt_p[:, cols], x_sb[:, cols], id_sb[:])

                xt_sb = xtpool.tile([P, N_STATE], dt, tag="xt_sb")
                nc.scalar.copy(xt_sb[:], xt_p[:])

                # y1 = x @ blockdiag(Gp, Gp), per block.
                y_p = ps_y.tile([P, N_STATE], dt, tag="y_ps")
                for b in range(N_BLK):
                    cols = slice(b * P, (b + 1) * P)
                    nc.tensor.matmul(y_p[:, cols], xt_sb[:, cols], gp2_sb[:],
                                     start=True, stop=True)

                # t1 = y1 * cosd (bcast over 16 heads); t2 = y1 * sinds.
                cos_v = trig_sb[:, t * P: t * P + H_DIM] \
                    .rearrange("p (o j) -> p o j", o=1) \
                    .broadcast_to([P, N_HEAD, H_DIM])
                sin_v = trig_sb[:, t * P + H_DIM: (t + 1) * P] \
                    .rearrange("p (o j) -> p o j", o=1) \
                    .broadcast_to([P, N_HEAD, H_DIM])
                y_v = y_p[:].rearrange("p (h j) -> p h j", h=N_HEAD)

                t1_sb = mixpool.tile([P, N_STATE], dt, tag="t1")
                t2_sb = mixpool.tile([P, N_STATE], dt, tag="t2")
                t1_v = t1_sb[:].rearrange("p (h j) -> p h j", h=N_HEAD)
                t2_v = t2_sb[:].rearrange("p (h j) -> p h j", h=N_HEAD)
                nc.vector.tensor_mul(t1_v, y_v, cos_v)
                nc.vector.tensor_mul(t2_v, y_v, sin_v)

                # out = t1 + half_swap(t2):  swap the two 32-halves of each
                # head of t2 (u<->v), realizing y2 = [-v | u] together with
                # the sign baked into the sin table.
                o_sb = opool.tile([P, N_STATE], dt, tag="o")
                o_v = o_sb[:].rearrange("p (h s j) -> p h s j", h=N_HEAD, s=2)
                t1_v4 = t1_sb[:].rearrange("p (h s j) -> p h s j",
                                           h=N_HEAD, s=2)
                t2_swap = t2_sb[:].rearrange("p (h s j) -> p h s j",
                                             h=N_HEAD, s=2)[:, :, ::-1, :]
                nc.gpsimd.tensor_add(o_v, t1_v4, t2_swap)

                nc.sync.dma_start(out[rows, :], o_sb[:])

    nc.compile()
    return nc


def _get_program():
    if "nc" not in _BUILD_CACHE:
        _BUILD_CACHE["nc"] = _build_program()
    return _BUILD_CACHE["nc"]


def _make_in_maps(inputs):
    x = np.ascontiguousarray(np.asarray(inputs["x"], np.float32))
    gp2, ident, trig = _build_constants(
        inputs["thetas"], inputs["theta_scale"], inputs["r_matrix"],
        inputs["inv_freq"], inputs["r_pairs"])
    in_maps = []
    for core in range(N_CORES):
        in_maps.append({
            "x": np.ascontiguousarray(
                x[core].reshape(TOKENS_PER_CORE, N_STATE)),
            "gp2": gp2, "ident": ident, "trig": trig,
        })
    return in_maps


def run(inputs):
    """Shard, execute on 8 cores, gather.  Returns (output, results)."""
    nc = _get_program()
    in_maps = _make_in_maps(inputs)
    res = run_bass_kernel_spmd(nc, in_maps, core_ids=list(range(N_CORES)))
    out = np.stack([res.results[c]["out"] for c in range(N_CORES)], axis=0)
    return out.reshape(B, S, N_STATE).astype(np.float32), res


def kernel(x, thetas, theta_scale, r_matrix, inv_freq, r_pairs, n_head):
    assert int(np.asarray(n_head)) == N_HEAD
    out, _ = run({
        "x": x, "thetas": thetas, "theta_scale": theta_scale,
        "r_matrix": r_matrix, "inv_freq": inv_freq, "r_pairs": r_pairs,
    })
    return out


# revision 5
# speedup vs baseline: 420.9595x; 420.9595x over previous
"""Trainium2 Bass kernel for nn_CombinedRotaryEmbedding.

Math
----
reference(x, ...) does, per (batch, seq, head) row r of length 64:
  1. 32 sequential Givens plane rotations -> r @ M_0 @ ... @ M_31
  2. r @ r_matrix
  3. RoPE mix with per-position sin/cos over even/odd channel pairs.

Steps 1-2 fold into ONE 64x64 matrix  Gtot = M_0 @ ... @ M_31 @ r_matrix
(computed on host in float64 from the tiny params).  We further permute
Gtot's columns (evens first, odds second) so that after  y1 = x @ Gp  the
head layout is [u | v] with u = x1 (even channels), v = x2 (odd channels)
— which is exactly the reference's output channel layout:
  out[..., 0:32]  = u*cos - v*sin
  out[..., 32:64] = u*sin + v*cos
The second RoPE operand  y2 = [-v | u]  is a sign-flipped half-swap of
y1, so no second matmul is needed: the elementwise stage reads y1 twice,
once with a half-swapped access pattern and a sign-folded sin table.

Device pipeline (per 128-token tile, fully unrolled, 32 tiles/core):
  ACT-DGE: DMA in x[128,1024]  ->  PE: 8x transpose(128x128) -> PSUM
  ACT: copy PSUM->SBUF  ->  PE: 8x matmul(lhsT=xT_blk, rhs=blockdiag(Gp,Gp))
  DVE: t1 = y1 * cos_bcast ; t2 = y1 * sin_signed_bcast   (PSUM src)
  POOL: out = t1 + half_swap(t2)   ->  SP-DGE: DMA out.
The full [4096, 128] cos|sin table is DMA'd to SBUF once at start.

Sharding: data-parallel, batch b -> core b (8 batches, 8 cores); all
params tiny and replicated.  Positions per core are s = 0..4095, so one
trig table is shared by all cores.
"""

import numpy as np

import concourse.bass as bass
import concourse.tile as tile
from concourse import bacc, mybir
from concourse.bass_utils import run_bass_kernel_spmd

# Problem constants (hardcoded per the task contract).
B, S, N_STATE, N_HEAD = 8, 4096, 1024, 16
H_DIM = N_STATE // N_HEAD        # 64
HALF = H_DIM // 2                # 32
N_CORES = 8
P = 128                          # partitions / tokens per tile
TOKENS_PER_CORE = S              # 4096
N_TILES = TOKENS_PER_CORE // P   # 32
N_BLK = N_STATE // P             # 8 channel blocks of 128 (2 heads each)

_BUILD_CACHE = {}


def _fold_g(angles, r_pairs, r_matrix):
    """Fold the Givens scan + r_matrix into one 64x64 (float64)."""
    g = np.eye(H_DIM, dtype=np.float64)
    eye = np.eye(H_DIM, dtype=np.float64)
    for k in range(angles.shape[0]):
        i, j = int(r_pairs[k, 0]), int(r_pairs[k, 1])
        c, sn = np.cos(angles[k]), np.sin(angles[k])
        m = eye.copy()
        # column i then column j, from the ORIGINAL basis columns —
        # replicates the reference's read-before-write .at[].set order
        # (also correct if i == j: the j write overwrites the i write).
        m[:, i] = c * eye[:, i] + sn * eye[:, j]
        m[:, j] = -sn * eye[:, i] + c * eye[:, j]
        g = g @ m
    g = g @ np.asarray(r_matrix, np.float64)
    return g


def _build_constants(thetas, theta_scale, r_matrix, inv_freq, r_pairs):
    """Host-side constant folding.

    Matches the reference's fp32 quantization points: the angle products
    (thetas * theta_scale, pos * inv_freq) are rounded to fp32 before the
    trig, as the fp32 reference does.
    """
    thetas = np.asarray(thetas, np.float32)
    theta_scale = np.asarray(theta_scale, np.float32)
    r_matrix = np.asarray(r_matrix, np.float32)
    inv_freq = np.asarray(inv_freq, np.float32)

    angles = (thetas * theta_scale[0]).astype(np.float32).astype(np.float64)
    gtot = _fold_g(angles, np.asarray(r_pairs), r_matrix)

    # Column permutation: evens first then odds -> y1 = [u | v] per head.
    perm = np.concatenate([np.arange(0, H_DIM, 2), np.arange(1, H_DIM, 2)])
    gp = gtot[:, perm].astype(np.float32)
    gp2 = np.zeros((P, P), np.float32)
    gp2[:H_DIM, :H_DIM] = gp
    gp2[H_DIM:, H_DIM:] = gp

    ident = np.eye(P, dtype=np.float32)

    pos = np.arange(S, dtype=np.float32)
    sinu32 = (pos[:, None] * inv_freq[None, :]).astype(np.float32)
    s64 = sinu32.astype(np.float64)
    cos_t = np.cos(s64).astype(np.float32)  # [S, 32]
    sin_t = np.sin(s64).astype(np.float32)
    # trig row layout per position: [cos|cos | +sin|-sin]  (128 wide).
    # cosd = [cos|cos]; sinds = [+sin|-sin]: t2 is computed in y1's layout
    # and later read half-swapped, so the coefficient that must land on
    # output half 0 (-sin) is stored in half 1 and vice versa.
    trig = np.concatenate([cos_t, cos_t, sin_t, -sin_t], axis=1)  # [S, 128]
    return gp2, ident, trig


def _build_program(repeat=1):
    """Build + compile the per-core Bass program (same NEFF on all cores).

    repeat > 1 wraps the whole 32-tile pipeline in a device-side For_i
    loop that recomputes the identical result `repeat` times — used only
    for benchmarking (amortizes host/tunnel dispatch overhead away).
    """
    nc = bacc.Bacc("TRN2", target_bir_lowering=False, debug=False,
                   num_devices=N_CORES)
    dt = mybir.dt.float32

    x = nc.dram_tensor("x", [TOKENS_PER_CORE, N_STATE], dt,
                       kind="ExternalInput").ap()
    gp2 = nc.dram_tensor("gp2", [P, P], dt, kind="ExternalInput").ap()
    ident = nc.dram_tensor("ident", [P, P], dt, kind="ExternalInput").ap()
    trig = nc.dram_tensor("trig", [S, P], dt, kind="ExternalInput").ap()
    out = nc.dram_tensor("out", [TOKENS_PER_CORE, N_STATE], dt,
                         kind="ExternalOutput").ap()

    with tile.TileContext(nc) as tc:
        with (
            tc.tile_pool(name="const", bufs=1) as cpool,
            tc.tile_pool(name="xin", bufs=4) as xpool,
            tc.tile_pool(name="xt", bufs=3) as xtpool,
            tc.tile_pool(name="mix", bufs=3) as mixpool,
            tc.tile_pool(name="outp", bufs=4) as opool,
            tc.tile_pool(name="ps_xt", bufs=2, space="PSUM") as ps_xt,
            tc.tile_pool(name="ps_y", bufs=2, space="PSUM") as ps_y,
        ):
            gp2_sb = cpool.tile([P, P], dt, tag="gp2")
            id_sb = cpool.tile([P, P], dt, tag="ident")
            nc.sync.dma_start(gp2_sb[:], gp2)
            nc.sync.dma_start(id_sb[:], ident)

            # Whole trig table, one DMA: tile t lives at columns 128t..128t+127.
            trig_sb = cpool.tile([P, N_TILES * P], dt, tag="trig")
            trig_dst = trig_sb[:].rearrange("p (t w) -> p t w", w=P)
            trig_src = trig.rearrange("(t p) w -> p t w", p=P)
            nc.sync.dma_start(trig_dst, trig_src)

            def body():
                for t in range(N_TILES):
                    _tile_body(nc, t, x, out, gp2_sb, id_sb, trig_sb,
                               xpool, xtpool, mixpool, opool, ps_xt, ps_y)

            if repeat == 1:
                body()
            else:
                with tc.For_i(0, repeat, 1,
                              hint_engines=(mybir.EngineType.PE,
                                            mybir.EngineType.DVE,
                                            mybir.EngineType.Activation,
                                            mybir.EngineType.Pool,
                                            mybir.EngineType.SP)):
                    body()

    nc.compile()
    return nc


def _tile_body(nc, t, x, out, gp2_sb, id_sb, trig_sb,
               xpool, xtpool, mixpool, opool, ps_xt, ps_y):
    dt = mybir.dt.float32
    rows = slice(t * P, (t + 1) * P)

    x_sb = xpool.tile([P, N_STATE], dt, tag="x")
    nc.scalar.dma_start(x_sb[:], x[rows, :])

    # Transpose 8 channel blocks: xt[c, tok] for c in block b.
    xt_p = ps_xt.tile([P, N_STATE], dt, tag="xt_ps")
    for b in range(N_BLK):
        cols = slice(b * P, (b + 1) * P)
        nc.tensor.transpose(xt_p[:, cols], x_sb[:, cols], id_sb[:])

    xt_sb = xtpool.tile([P, N_STATE], dt, tag="xt_sb")
    nc.scalar.copy(xt_sb[:], xt_p[:])

    # y1 = x @ blockdiag(Gp, Gp), per block.
    y_p = ps_y.tile([P, N_STATE], dt, tag="y_ps")
    for b in range(N_BLK):
        cols = slice(b * P, (b + 1) * P)
        nc.tensor.matmul(y_p[:, cols], xt_sb[:, cols], gp2_sb[:],
                         start=True, stop=True)

    # t1 = y1 * cosd (bcast over 16 heads); t2 = y1 * sinds.
    cos_v = trig_sb[:, t * P: t * P + H_DIM] \
        .rearrange("p (o j) -> p o j", o=1) \
        .broadcast_to([P, N_HEAD, H_DIM])
    sin_v = trig_sb[:, t * P + H_DIM: (t + 1) * P] \
        .rearrange("p (o j) -> p o j", o=1) \
        .broadcast_to([P, N_HEAD, H_DIM])
    y_v = y_p[:].rearrange("p (h j) -> p h j", h=N_HEAD)

    t1_sb = mixpool.tile([P, N_STATE], dt, tag="t1")
    t2_sb = mixpool.tile([P, N_STATE], dt, tag="t2")
    t1_v = t1_sb[:].rearrange("p (h j) -> p h j", h=N_HEAD)
    t2_v = t2_sb[:].rearrange("p (h j) -> p h j", h=N_HEAD)
    nc.vector.tensor_mul(t1_v, y_v, cos_v)
    nc.vector.tensor_mul(t2_v, y_v, sin_v)

    # out = t1 + half_swap(t2):  swap the two 32-halves of each
    # head of t2 (u<->v), realizing y2 = [-v | u] together with
    # the sign baked into the sin table.
    o_sb = opool.tile([P, N_STATE], dt, tag="o")
    o_v = o_sb[:].rearrange("p (h s j) -> p h s j", h=N_HEAD, s=2)
    t1_v4 = t1_sb[:].rearrange("p (h s j) -> p h s j", h=N_HEAD, s=2)
    t2_swap = t2_sb[:].rearrange("p (h s j) -> p h s j",
                                 h=N_HEAD, s=2)[:, :, ::-1, :]
    nc.gpsimd.tensor_add(o_v, t1_v4, t2_swap)

    nc.sync.dma_start(out[rows, :], o_sb[:])


def _get_program(repeat=1):
    key = ("nc", repeat)
    if key not in _BUILD_CACHE:
        _BUILD_CACHE[key] = _build_program(repeat)
    return _BUILD_CACHE[key]


def _make_in_maps(inputs):
    x = np.ascontiguousarray(np.asarray(inputs["x"], np.float32))
    gp2, ident, trig = _build_constants(
        inputs["thetas"], inputs["theta_scale"], inputs["r_matrix"],
        inputs["inv_freq"], inputs["r_pairs"])
    in_maps = []
    for core in range(N_CORES):
        in_maps.append({
            "x": np.ascontiguousarray(
                x[core].reshape(TOKENS_PER_CORE, N_STATE)),
            "gp2": gp2, "ident": ident, "trig": trig,
        })
    return in_maps


def run(inputs):
    """Shard, execute on 8 cores, gather.  Returns (output, results)."""
    nc = _get_program()
    in_maps = _make_in_maps(inputs)
    res = run_bass_kernel_spmd(nc, in_maps, core_ids=list(range(N_CORES)))
    out = np.stack([res.results[c]["out"] for c in range(N_CORES)], axis=0)
    return out.reshape(B, S, N_STATE).astype(np.float32), res


def kernel(x, thetas, theta_scale, r_matrix, inv_freq, r_pairs, n_head):
    assert int(np.asarray(n_head)) == N_HEAD
    out, _ = run({
        "x": x, "thetas": thetas, "theta_scale": theta_scale,
        "r_matrix": r_matrix, "inv_freq": inv_freq, "r_pairs": r_pairs,
    })
    return out


# revision 6
# speedup vs baseline: 841.5226x; 1.9991x over previous
"""Trainium2 Bass kernel for nn_CombinedRotaryEmbedding.

Math
----
reference(x, ...) does, per (batch, seq, head) row r of length 64:
  1. 32 sequential Givens plane rotations -> r @ M_0 @ ... @ M_31
  2. r @ r_matrix
  3. RoPE mix with per-position sin/cos over even/odd channel pairs.

Steps 1-2 fold into ONE 64x64 matrix  Gtot = M_0 @ ... @ M_31 @ r_matrix
(computed on host in float64 from the tiny params).  We further permute
Gtot's columns (evens first, odds second) so that after  y1 = x @ Gp  the
head layout is [u | v] with u = x1 (even channels), v = x2 (odd channels)
— which is exactly the reference's output channel layout:
  out[..., 0:32]  = u*cos - v*sin
  out[..., 32:64] = u*sin + v*cos
The second RoPE operand  y2 = [-v | u]  is a sign-flipped half-swap of
y1, so no second matmul is needed: the elementwise stage reads y1 twice,
once with a half-swapped access pattern and a sign-folded sin table.

Device pipeline (per 128-token tile, fully unrolled, 32 tiles/core):
  ACT-DGE: DMA in x[128,1024]  ->  PE: 8x transpose(128x128) -> PSUM
  ACT: copy PSUM->SBUF  ->  PE: 8x matmul(lhsT=xT_blk, rhs=blockdiag(Gp,Gp))
  DVE: t1 = y1 * cos_bcast ; t2 = y1 * sin_signed_bcast   (PSUM src)
  POOL: out = t1 + half_swap(t2)   ->  SP-DGE: DMA out.
The full [4096, 128] cos|sin table is DMA'd to SBUF once at start.

Sharding: data-parallel, batch b -> core b (8 batches, 8 cores); all
params tiny and replicated.  Positions per core are s = 0..4095, so one
trig table is shared by all cores.
"""

import numpy as np

import concourse.bass as bass
import concourse.tile as tile
from concourse import bacc, mybir
from concourse.bass_utils import run_bass_kernel_spmd

# Problem constants (hardcoded per the task contract).
B, S, N_STATE, N_HEAD = 8, 4096, 1024, 16
H_DIM = N_STATE // N_HEAD        # 64
HALF = H_DIM // 2                # 32
N_CORES = 8
P = 128                          # partitions / tokens per tile
TOKENS_PER_CORE = S              # 4096
N_TILES = TOKENS_PER_CORE // P   # 32
N_BLK = N_STATE // P             # 8 channel blocks of 128 (2 heads each)

_BUILD_CACHE = {}


def _fold_g(angles, r_pairs, r_matrix):
    """Fold the Givens scan + r_matrix into one 64x64 (float64)."""
    g = np.eye(H_DIM, dtype=np.float64)
    eye = np.eye(H_DIM, dtype=np.float64)
    for k in range(angles.shape[0]):
        i, j = int(r_pairs[k, 0]), int(r_pairs[k, 1])
        c, sn = np.cos(angles[k]), np.sin(angles[k])
        m = eye.copy()
        # column i then column j, from the ORIGINAL basis columns —
        # replicates the reference's read-before-write .at[].set order
        # (also correct if i == j: the j write overwrites the i write).
        m[:, i] = c * eye[:, i] + sn * eye[:, j]
        m[:, j] = -sn * eye[:, i] + c * eye[:, j]
        g = g @ m
    g = g @ np.asarray(r_matrix, np.float64)
    return g


def _build_constants(thetas, theta_scale, r_matrix, inv_freq, r_pairs):
    """Host-side constant folding.

    Matches the reference's fp32 quantization points: the angle products
    (thetas * theta_scale, pos * inv_freq) are rounded to fp32 before the
    trig, as the fp32 reference does.
    """
    thetas = np.asarray(thetas, np.float32)
    theta_scale = np.asarray(theta_scale, np.float32)
    r_matrix = np.asarray(r_matrix, np.float32)
    inv_freq = np.asarray(inv_freq, np.float32)

    angles = (thetas * theta_scale[0]).astype(np.float32).astype(np.float64)
    gtot = _fold_g(angles, np.asarray(r_pairs), r_matrix)

    # Column permutation: evens first then odds -> y1 = [u | v] per head.
    perm = np.concatenate([np.arange(0, H_DIM, 2), np.arange(1, H_DIM, 2)])
    gp = gtot[:, perm].astype(np.float32)
    gp2 = np.zeros((P, P), np.float32)
    gp2[:H_DIM, :H_DIM] = gp
    gp2[H_DIM:, H_DIM:] = gp

    ident = np.eye(P, dtype=np.float32)

    pos = np.arange(S, dtype=np.float32)
    sinu32 = (pos[:, None] * inv_freq[None, :]).astype(np.float32)
    s64 = sinu32.astype(np.float64)
    cos_t = np.cos(s64).astype(np.float32)  # [S, 32]
    sin_t = np.sin(s64).astype(np.float32)
    # trig row layout per position: [cos|cos | +sin|-sin]  (128 wide).
    # cosd = [cos|cos]; sinds = [+sin|-sin]: t2 is computed in y1's layout
    # and later read half-swapped, so the coefficient that must land on
    # output half 0 (-sin) is stored in half 1 and vice versa.
    trig = np.concatenate([cos_t, cos_t, sin_t, -sin_t], axis=1)  # [S, 128]
    return gp2, ident, trig


def _build_program(repeat=1):
    """Build + compile the per-core Bass program (same NEFF on all cores).

    repeat > 1 wraps the whole 32-tile pipeline in a device-side For_i
    loop that recomputes the identical result `repeat` times — used only
    for benchmarking (amortizes host/tunnel dispatch overhead away).
    """
    nc = bacc.Bacc("TRN2", target_bir_lowering=False, debug=False,
                   num_devices=N_CORES)
    dt = mybir.dt.float32

    x = nc.dram_tensor("x", [TOKENS_PER_CORE, N_STATE], dt,
                       kind="ExternalInput").ap()
    gp2 = nc.dram_tensor("gp2", [P, P], dt, kind="ExternalInput").ap()
    ident = nc.dram_tensor("ident", [P, P], dt, kind="ExternalInput").ap()
    trig = nc.dram_tensor("trig", [S, P], dt, kind="ExternalInput").ap()
    out = nc.dram_tensor("out", [TOKENS_PER_CORE, N_STATE], dt,
                         kind="ExternalOutput").ap()

    with tile.TileContext(nc) as tc:
        with (
            tc.tile_pool(name="const", bufs=1) as cpool,
            tc.tile_pool(name="xin", bufs=4) as xpool,
            tc.tile_pool(name="xt", bufs=3) as xtpool,
            tc.tile_pool(name="mix", bufs=3) as mixpool,
            tc.tile_pool(name="outp", bufs=4) as opool,
            tc.tile_pool(name="ps_xt", bufs=2, space="PSUM") as ps_xt,
            tc.tile_pool(name="ps_y", bufs=2, space="PSUM") as ps_y,
        ):
            gp2_sb = cpool.tile([P, P], dt, tag="gp2")
            id_sb = cpool.tile([P, P], dt, tag="ident")
            nc.sync.dma_start(gp2_sb[:], gp2)
            nc.sync.dma_start(id_sb[:], ident)

            # Whole trig table, one DMA: tile t lives at columns 128t..128t+127.
            trig_sb = cpool.tile([P, N_TILES * P], dt, tag="trig")
            trig_dst = trig_sb[:].rearrange("p (t w) -> p t w", w=P)
            trig_src = trig.rearrange("(t p) w -> p t w", p=P)
            nc.sync.dma_start(trig_dst, trig_src)

            # PE warmup: ~6 us of back-to-back bf16 matmuls on zeros.  The
            # HAM clock gate keeps PE at 1.2 GHz until it sees a sustained
            # busy window; transpose-mode ops never count as busy, so
            # without this the TR/MM interleave stays throttled forever.
            # Once warm, the pipeline's PE gaps are well under the ~3.4 us
            # idle window, so the warm state persists.
            warm_bf = cpool.tile([P, 640], mybir.dt.bfloat16, tag="warmsrc")
            nc.vector.memset(warm_bf[:], 0.0)
            y_warm = ps_y.tile([P, N_STATE], dt, tag="y_ps")
            for _ in range(28):
                nc.tensor.matmul(y_warm[:, :512], warm_bf[:, :128],
                                 warm_bf[:, 128:640], start=True, stop=True)

            def body():
                for t in range(N_TILES):
                    _tile_body(nc, t, x, out, gp2_sb, id_sb, trig_sb,
                               xpool, xtpool, mixpool, opool, ps_xt, ps_y)

            if repeat == 1:
                body()
            else:
                with tc.For_i(0, repeat, 1,
                              hint_engines=(mybir.EngineType.PE,
                                            mybir.EngineType.DVE,
                                            mybir.EngineType.Activation,
                                            mybir.EngineType.Pool,
                                            mybir.EngineType.SP)):
                    body()

    nc.compile()
    return nc


def _tile_body(nc, t, x, out, gp2_sb, id_sb, trig_sb,
               xpool, xtpool, mixpool, opool, ps_xt, ps_y):
    dt = mybir.dt.float32
    rows = slice(t * P, (t + 1) * P)

    x_sb = xpool.tile([P, N_STATE], dt, tag="x")
    nc.scalar.dma_start(x_sb[:], x[rows, :])

    # Transpose 8 channel blocks: xt[c, tok] for c in block b.
    xt_p = ps_xt.tile([P, N_STATE], dt, tag="xt_ps")
    for b in range(N_BLK):
        cols = slice(b * P, (b + 1) * P)
        nc.tensor.transpose(xt_p[:, cols], x_sb[:, cols], id_sb[:])

    xt_sb = xtpool.tile([P, N_STATE], dt, tag="xt_sb")
    nc.scalar.copy(xt_sb[:], xt_p[:])

    # y1 = x @ blockdiag(Gp, Gp), per block.
    y_p = ps_y.tile([P, N_STATE], dt, tag="y_ps")
    for b in range(N_BLK):
        cols = slice(b * P, (b + 1) * P)
        nc.tensor.matmul(y_p[:, cols], xt_sb[:, cols], gp2_sb[:],
                         start=True, stop=True)

    # t1 = y1 * cosd (bcast over 16 heads); t2 = y1 * sinds.
    cos_v = trig_sb[:, t * P: t * P + H_DIM] \
        .rearrange("p (o j) -> p o j", o=1) \
        .broadcast_to([P, N_HEAD, H_DIM])
    sin_v = trig_sb[:, t * P + H_DIM: (t + 1) * P] \
        .rearrange("p (o j) -> p o j", o=1) \
        .broadcast_to([P, N_HEAD, H_DIM])
    y_v = y_p[:].rearrange("p (h j) -> p h j", h=N_HEAD)

    t1_sb = mixpool.tile([P, N_STATE], dt, tag="t1")
    t2_sb = mixpool.tile([P, N_STATE], dt, tag="t2")
    t1_v = t1_sb[:].rearrange("p (h j) -> p h j", h=N_HEAD)
    t2_v = t2_sb[:].rearrange("p (h j) -> p h j", h=N_HEAD)
    nc.vector.tensor_mul(t1_v, y_v, cos_v)
    nc.vector.tensor_mul(t2_v, y_v, sin_v)

    # out = t1 + half_swap(t2):  swap the two 32-halves of each
    # head of t2 (u<->v), realizing y2 = [-v | u] together with
    # the sign baked into the sin table.
    o_sb = opool.tile([P, N_STATE], dt, tag="o")
    o_v = o_sb[:].rearrange("p (h s j) -> p h s j", h=N_HEAD, s=2)
    t1_v4 = t1_sb[:].rearrange("p (h s j) -> p h s j", h=N_HEAD, s=2)
    t2_swap = t2_sb[:].rearrange("p (h s j) -> p h s j",
                                 h=N_HEAD, s=2)[:, :, ::-1, :]
    nc.gpsimd.tensor_add(o_v, t1_v4, t2_swap)

    nc.sync.dma_start(out[rows, :], o_sb[:])


def _get_program(repeat=1):
    key = ("nc", repeat)
    if key not in _BUILD_CACHE:
        _BUILD_CACHE[key] = _build_program(repeat)
    return _BUILD_CACHE[key]


def _make_in_maps(inputs):
    x = np.ascontiguousarray(np.asarray(inputs["x"], np.float32))
    gp2, ident, trig = _build_constants(
        inputs["thetas"], inputs["theta_scale"], inputs["r_matrix"],
        inputs["inv_freq"], inputs["r_pairs"])
    in_maps = []
    for core in range(N_CORES):
        in_maps.append({
            "x": np.ascontiguousarray(
                x[core].reshape(TOKENS_PER_CORE, N_STATE)),
            "gp2": gp2, "ident": ident, "trig": trig,
        })
    return in_maps


def run(inputs):
    """Shard, execute on 8 cores, gather.  Returns (output, results)."""
    nc = _get_program()
    in_maps = _make_in_maps(inputs)
    res = run_bass_kernel_spmd(nc, in_maps, core_ids=list(range(N_CORES)))
    out = np.stack([res.results[c]["out"] for c in range(N_CORES)], axis=0)
    return out.reshape(B, S, N_STATE).astype(np.float32), res


def kernel(x, thetas, theta_scale, r_matrix, inv_freq, r_pairs, n_head):
    assert int(np.asarray(n_head)) == N_HEAD
    out, _ = run({
        "x": x, "thetas": thetas, "theta_scale": theta_scale,
        "r_matrix": r_matrix, "inv_freq": inv_freq, "r_pairs": r_pairs,
    })
    return out
